# revision 1
# baseline (speedup 1.0000x reference)
"""Trainium2 Bass kernel for nn_BallQLoss: PointNet++-style ball query +
grouping + L1 mask loss, sharded over 8 NeuronCores.

Per core: one (batch, row-half) shard -> 2048 query rows x 4096 candidate
columns. Pipeline per 128-row block:
  PE:   P4[n,j] = 2*dot(pc_n,pc_j) - sq_j           (K=4 matmul, f32)
  ACT:  S = sign(P4 + (r^2 - sq_n))                 (+1 in-ball, -1 out, 0 tie)
  DVE:  keyed = S * (N - j); top-16 via max8/match_replace/max8
        -> first 16 in-ball indices in ascending-j order, padded w/ first
  DMA:  wrap idx to SWDGE layout (DRAM round trip), dma_gather mask rows
  DVE:  sum |mask[n,c] - mask[idx,c]| over (slot, c) per row
Final: per-core scalar partial via ones-matmul partition reduce; host sums
partials and divides by (B*N*K).
"""
import os
import sys

import numpy as np

try:
    import concourse.bass as bass
except ImportError:
    sys.path.insert(0, '/opt/trn_rl_repo')
    import concourse.bass as bass

import concourse.mybir as mybir
import concourse.tile as tile
from concourse import bacc
from concourse.bass_utils import run_bass_kernel_spmd

f32 = mybir.dt.float32
f16 = mybir.dt.float16
bf16 = mybir.dt.bfloat16
u16 = mybir.dt.uint16
i16 = mybir.dt.int16
i32 = mybir.dt.int32
KDIM = 21  # 6 hi/mid/lo cross pairs x 3 dims + 3 split -sq rows
# f16 descending key table: nj[j] = bitcast_f16(NJ_BASE - j); consecutive f16
# ULPs are consecutive integer bit patterns, so j = NJ_BASE - bits(v).
NJ_BASE = 27648  # bits of f16(4096.0)

B = 4            # batches
N = 4096         # points per batch
C = 30           # mask channels
KN = 16          # neighbors per query
R2 = np.float32(0.2) * np.float32(0.2)
NCORES = 8
ROWS = 2048      # query rows per core (half a batch)
NBLK = ROWS // 128
NF = N // 512    # 512-wide column tiles per block

_PROGRAM = None


def _build_program():
    nc = bacc.Bacc("TRN2", target_bir_lowering=False, debug=False)

    lhsT_d = nc.dram_tensor("lhsT", [KDIM, ROWS], bf16, kind="ExternalInput")
    rhs_d = nc.dram_tensor("rhs", [KDIM, N], bf16, kind="ExternalInput")
    nthr_d = nc.dram_tensor("nthr", [128, NBLK], f32, kind="ExternalInput")
    nj_d = nc.dram_tensor("nj", [N], f16, kind="ExternalInput")
    maskp_d = nc.dram_tensor("maskp", [N, 64], f32, kind="ExternalInput")
    own_d = nc.dram_tensor("own", [ROWS, C], f32, kind="ExternalInput")
    partial_d = nc.dram_tensor("partial", [1, 1], f32, kind="ExternalOutput")

    with tile.TileContext(nc) as tc:
        with (
            tc.tile_pool(name="const", bufs=1) as const_pool,
            tc.tile_pool(name="psum", bufs=6, space="PSUM") as psum_pool,
            tc.tile_pool(name="psumf", bufs=1, space="PSUM") as psumf_pool,
            tc.tile_pool(name="sbS", bufs=4) as s_pool,
            tc.tile_pool(name="sbK", bufs=3) as k_pool,
            tc.tile_pool(name="small", bufs=6) as small_pool,
            tc.tile_pool(name="gat", bufs=6) as gat_pool,
        ):
            lhsT = const_pool.tile([KDIM, ROWS], bf16)
            nc.sync.dma_start(lhsT[:], lhsT_d[:])
            rhs = const_pool.tile([KDIM, N], bf16)
            nc.sync.dma_start(rhs[:], rhs_d[:])
            nthr = const_pool.tile([128, NBLK], f32)
            nc.sync.dma_start(nthr[:], nthr_d[:])
            nj = const_pool.tile([128, N], f16)
            nc.sync.dma_start(nj[:], bass.AP(nj_d, 0, [[0, 128], [1, N]]))
            acc_all = const_pool.tile([128, NBLK], f32)

            def half_idx(v8, v0col, tag):
                """first-in-ball pad fix + f16-ULP-bitcast index recovery for
                8 slots: idx = NJ_BASE_BITS - bits(v>0 ? v : v0)."""
                m = small_pool.tile([128, 8], f16, tag=f"m{tag}")
                nc.vector.tensor_scalar(out=m[:], in0=v8, scalar1=0.0,
                                        scalar2=None, op0=mybir.AluOpType.is_gt)
                negv0 = small_pool.tile([128, 1], f32, tag=f"negv0{tag}")
                nc.vector.tensor_scalar(out=negv0[:], in0=v0col, scalar1=-1.0,
                                        scalar2=None, op0=mybir.AluOpType.mult)
                pfix = small_pool.tile([128, 8], f16, tag=f"pfix{tag}")
                nc.vector.tensor_scalar(out=pfix[:], in0=m[:], scalar1=1.0,
                                        scalar2=negv0[:, 0:1],
                                        op0=mybir.AluOpType.subtract,
                                        op1=mybir.AluOpType.mult)
                vfix = small_pool.tile([128, 8], f16, tag=f"vfix{tag}")
                nc.vector.tensor_tensor(out=vfix[:], in0=v8, in1=m[:],
                                        op=mybir.AluOpType.mult)
                nc.vector.tensor_tensor(out=vfix[:], in0=vfix[:], in1=pfix[:],
                                        op=mybir.AluOpType.add)
                bits = small_pool.tile([128, 8], f32, tag=f"bits{tag}")
                nc.vector.tensor_copy(bits[:], vfix[:].bitcast(u16))
                idxi = small_pool.tile([128, 8], i32, tag=f"idxi{tag}")
                nc.vector.tensor_scalar(out=idxi[:], in0=bits[:],
                                        scalar1=-1.0, scalar2=float(NJ_BASE),
                                        op0=mybir.AluOpType.mult,
                                        op1=mybir.AluOpType.add)
                return idxi

            prev = None  # (G, own, blk) pending |diff| reduce, 1-block lag

            def flush_prev():
                nonlocal prev
                if prev is None:
                    return
                G, own, pblk = prev
                D = gat_pool.tile([128, KN, C], f32, tag="D")
                nc.vector.tensor_tensor(
                    out=D[:], in0=G[:],
                    in1=own[:].unsqueeze(1).broadcast_to((128, KN, C)),
                    op=mybir.AluOpType.subtract)
                nc.vector.reduce_sum(acc_all[:, pblk:pblk + 1], D[:],
                                     mybir.AxisListType.XY,
                                     apply_absolute_value=True)
                prev = None

            for blk in range(NBLK):
                S = s_pool.tile([128, N], f16)
                keyed = k_pool.tile([128, N], f16)
                for f in range(NF):
                    fs = slice(f * 512, (f + 1) * 512)
                    p = psum_pool.tile([128, 512], f32)
                    nc.tensor.matmul(p[:], lhsT[:, blk * 128:(blk + 1) * 128],
                                     rhs[:, fs])
                    nc.scalar.activation(S[:, fs], p[:],
                                         mybir.ActivationFunctionType.Sign,
                                         bias=nthr[:, blk:blk + 1], scale=1.0)
                    nc.vector.tensor_tensor(out=keyed[:, fs], in0=S[:, fs],
                                            in1=nj[:, fs],
                                            op=mybir.AluOpType.mult)

                v16 = small_pool.tile([128, KN], f16)
                nc.vector.max(v16[:, 0:8], keyed[:])
                # slots 0-7 gather issues overlap match_replace/max8 #2
                idxiA = half_idx(v16[:, 0:8], v16[:, 0:1], "A")
                G = gat_pool.tile([128, KN, C], f32)
                own = small_pool.tile([128, C], f32, tag="own")
                nc.sync.dma_start(own[:], own_d[blk * 128:(blk + 1) * 128, :])
                for s in range(8):
                    nc.gpsimd.indirect_dma_start(
                        G[:, s, :], None, maskp_d[:],
                        bass.IndirectOffsetOnAxis(ap=idxiA[:, s:s + 1], axis=0))

                nc.vector.match_replace(keyed[:], v16[:, 0:8], keyed[:],
                                        -60000.0)
                flush_prev()
                nc.vector.max(v16[:, 8:16], keyed[:])
                idxiB = half_idx(v16[:, 8:16], v16[:, 0:1], "B")
                for s in range(8):
                    nc.gpsimd.indirect_dma_start(
                        G[:, 8 + s, :], None, maskp_d[:],
                        bass.IndirectOffsetOnAxis(ap=idxiB[:, s:s + 1], axis=0))

                prev = (G, own, blk)
            flush_prev()

            rowtot = const_pool.tile([128, 1], f32)
            nc.vector.reduce_sum(rowtot[:], acc_all[:], mybir.AxisListType.X)
            ones = const_pool.tile([128, 1], f32)
            nc.vector.memset(ones[:], 1.0)
            ptot = psumf_pool.tile([1, 1], f32)
            nc.tensor.matmul(ptot[:], rowtot[:], ones[:])
            tot = const_pool.tile([1, 1], f32)
            nc.vector.tensor_copy(tot[:], ptot[:])
            nc.sync.dma_start(partial_d[:], tot[:])

    nc.compile()
    return nc


def _get_program():
    global _PROGRAM
    if _PROGRAM is None:
        _PROGRAM = _build_program()
    return _PROGRAM


try:
    import ml_dtypes
    _BF = ml_dtypes.bfloat16
except ImportError:
    _BF = None


def _split3(v):
    """f32 -> (hi, mid, lo) bf16 triplet with hi+mid+lo ~ v to ~2^-25 rel."""
    v = np.asarray(v, np.float32)
    h = v.astype(_BF)
    r = v - h.astype(np.float32)
    m = r.astype(_BF)
    l = (r - m.astype(np.float32)).astype(_BF)
    return h, m, l


def _make_in_maps(pc: np.ndarray, mask: np.ndarray):
    pc = np.asarray(pc, np.float32)
    mask = np.asarray(mask, np.float32)
    nj = (NJ_BASE - np.arange(N)).astype(np.uint16).view(np.float16)
    in_maps = []
    for core in range(NCORES):
        b, h = divmod(core, 2)
        rows = slice(h * ROWS, (h + 1) * ROWS)
        pcb = pc[b]                       # (N, 3)
        sq = np.sum(pcb * pcb, axis=1)    # (N,)
        # 3-way bf16 split of 2*pc_n (rows) and pc_j (cols); P4 accumulates
        # the 6 dominant cross products + split -sq_j rows in f32 PSUM.
        xh, xm, xl = _split3(2.0 * pcb[rows])
        yh, ym, yl = _split3(pcb)
        sh, sm, sl = _split3(sq)
        ones = np.ones((ROWS,), _BF)
        lhsT = np.stack([r for a, _ in ((xh, yh), (xh, ym), (xm, yh),
                                        (xh, yl), (xl, yh), (xm, ym))
                         for r in (a[:, 0], a[:, 1], a[:, 2])]
                        + [ones, ones, ones], axis=0)
        rhs = np.stack([r for _, bb in ((xh, yh), (xh, ym), (xm, yh),
                                        (xh, yl), (xl, yh), (xm, ym))
                        for r in (bb[:, 0], bb[:, 1], bb[:, 2])]
                       + [-sh, -sm, -sl], axis=0)
        nthr = (R2 - sq[rows]).reshape(NBLK, 128).T.copy()
        maskp = np.zeros((N, 64), np.float32)
        maskp[:, :C] = mask[b]
        in_maps.append({"lhsT": np.ascontiguousarray(lhsT),
                        "rhs": np.ascontiguousarray(rhs),
                        "nthr": np.ascontiguousarray(nthr),
                        "nj": nj,
                        "maskp": maskp,
                        "own": np.ascontiguousarray(mask[b][rows])})
    return in_maps


def _run(pc, mask, trace=False):
    nc = _get_program()
    in_maps = _make_in_maps(pc, mask)
    res = run_bass_kernel_spmd(nc, in_maps, list(range(NCORES)), trace=trace)
    total = sum(float(r["partial"][0, 0]) for r in res.results)
    loss = np.float32(total / (B * N * KN))
    return np.asarray(loss, dtype=np.float32), res


def kernel(pc, mask):
    loss, _ = _run(pc, mask)
    return loss



# revision 6
# speedup vs baseline: 1.1853x; 1.1853x over previous
"""Trainium2 Bass kernel for nn_BallQLoss: PointNet++-style ball query +
grouping + L1 mask loss, sharded over 8 NeuronCores.

Per core: one (batch, row-half) shard -> 2048 query rows x 4096 candidate
columns. Pipeline per 128-row block:
  PE:   P4[n,j] = 2*dot(pc_n,pc_j) - sq_j        (K=21 matmul, bf16 splits)
  ACT:  S = sign(P4 + (r^2 - sq_n))              (2 x [128,2048] PSUM reads)
  DVE:  keyed = S * nj  (nj[j] = f16 ULP-coded descending index key)
        top-16 via max8 / (keyed < v8min)*keyed / max8
        pad fix + f16-ULP-bitcast index recovery -> idxi [128,16] i32
  POOL: ONE batched indirect gather [128,16] -> G [128,16,32] f16
  DVE:  D = G - own; acc[:, blk] = sum |D|  (fused abs_max accum)
Final: partial [128, NBLK] f32 per core, host sums and divides by B*N*K.
"""
import os
import sys

import numpy as np

try:
    import concourse.bass as bass
except ImportError:
    sys.path.insert(0, '/opt/trn_rl_repo')
    import concourse.bass as bass

import concourse.mybir as mybir
import concourse.tile as tile
from concourse import bacc
from concourse.bass_utils import run_bass_kernel_spmd

f32 = mybir.dt.float32
f16 = mybir.dt.float16
bf16 = mybir.dt.bfloat16
u16 = mybir.dt.uint16
i32 = mybir.dt.int32
KDIM = 21  # 6 hi/mid/lo cross pairs x 3 dims + 3 split -sq rows
# f16 descending key table: nj[j] = bitcast_f16(NJ_BASE - j); consecutive f16
# ULPs are consecutive integer bit patterns, so j = NJ_BASE - bits(v).
NJ_BASE = 27648  # bits of f16(4096.0)

B = 4            # batches
N = 4096         # points per batch
C = 30           # mask channels
CP = 32          # padded channel count (contiguous 64B gather rows)
KN = 16          # neighbors per query
R2 = np.float32(0.2) * np.float32(0.2)
NCORES = 8
ROWS = 2048      # query rows per core (half a batch)
NBLK = ROWS // 128
BATCH_GATHER = os.environ.get("BATCH_GATHER", "0") == "1"

_PROGRAM = None


def _build_program():
    nc = bacc.Bacc("TRN2", target_bir_lowering=False, debug=False)

    lhsT_d = nc.dram_tensor("lhsT", [KDIM, ROWS], bf16, kind="ExternalInput")
    rhs_d = nc.dram_tensor("rhs", [KDIM, N], bf16, kind="ExternalInput")
    nthr_d = nc.dram_tensor("nthr", [128, NBLK], f32, kind="ExternalInput")
    nj_d = nc.dram_tensor("nj", [N], f16, kind="ExternalInput")
    maskp_d = nc.dram_tensor("maskp", [N, CP], f16, kind="ExternalInput")
    own_d = nc.dram_tensor("own", [ROWS, CP], f16, kind="ExternalInput")
    partial_d = nc.dram_tensor("partial", [128, NBLK], f32,
                               kind="ExternalOutput")

    with tile.TileContext(nc) as tc:
        with (
            tc.tile_pool(name="const", bufs=1) as const_pool,
            tc.tile_pool(name="psum", bufs=2, space="PSUM") as psum_pool,
            tc.tile_pool(name="sbS", bufs=2) as s_pool,
            tc.tile_pool(name="sbK", bufs=2) as k_pool,
            tc.tile_pool(name="sbK2", bufs=2) as k2_pool,
            tc.tile_pool(name="small", bufs=3) as small_pool,
            tc.tile_pool(name="gat", bufs=3) as gat_pool,
        ):
            lhsT = const_pool.tile([KDIM, ROWS], bf16)
            nc.sync.dma_start(lhsT[:], lhsT_d[:])
            rhs = const_pool.tile([KDIM, N], bf16)
            nc.sync.dma_start(rhs[:], rhs_d[:])
            nthr = const_pool.tile([128, NBLK], f32)
            nc.sync.dma_start(nthr[:], nthr_d[:])
            nj = const_pool.tile([128, N], f16)
            nc.sync.dma_start(nj[:], bass.AP(nj_d, 0, [[0, 128], [1, N]]))
            acc_all = const_pool.tile([128, NBLK], f32)

            prev = None  # (G, own, blk) pending |diff| reduce, 1-block lag

            def flush_prev():
                nonlocal prev
                if prev is None:
                    return
                G, own, pblk = prev
                D = gat_pool.tile([128, KN, C], f16, tag="D")
                nc.vector.tensor_tensor(
                    out=D[:], in0=G[:, :, 0:C],
                    in1=own[:, 0:C].unsqueeze(1).broadcast_to((128, KN, C)),
                    op=mybir.AluOpType.subtract)
                nc.vector.reduce_sum(acc_all[:, pblk:pblk + 1], D[:],
                                     mybir.AxisListType.XY,
                                     apply_absolute_value=True)
                prev = None

            for blk in range(NBLK):
                S = s_pool.tile([128, N], f16)
                keyed = k_pool.tile([128, N], f16)
                for h in range(2):
                    hs = slice(h * 2048, (h + 1) * 2048)
                    p = psum_pool.tile([128, 2048], f32, tag="p")
                    for f in range(4):
                        fs = slice(f * 512, (f + 1) * 512)
                        nc.tensor.matmul(
                            p[:, fs], lhsT[:, blk * 128:(blk + 1) * 128],
                            rhs[:, h * 2048 + f * 512:
                                h * 2048 + (f + 1) * 512])
                    nc.scalar.activation(S[:, hs], p[:],
                                         mybir.ActivationFunctionType.Sign,
                                         bias=nthr[:, blk:blk + 1], scale=1.0)
                    nc.vector.tensor_tensor(out=keyed[:, hs], in0=S[:, hs],
                                            in1=nj[:, hs],
                                            op=mybir.AluOpType.mult)

                v16 = small_pool.tile([128, KN], f16, tag="v16")
                nc.vector.max(v16[:, 0:8], keyed[:])
                keyed2 = k2_pool.tile([128, N], f16)
                # mask out the top-8 (strictly-less keeps everything below the
                # 8th largest; keys are distinct so this equals match_replace)
                nc.vector.scalar_tensor_tensor(
                    out=keyed2[:], in0=keyed[:], scalar=v16[:, 7:8],
                    in1=keyed[:], op0=mybir.AluOpType.is_lt,
                    op1=mybir.AluOpType.mult)
                nc.vector.max(v16[:, 8:16], keyed2[:])

                # pad fix: slots with v<=0 get v0 (first in-ball key); then
                # idx = NJ_BASE - bits(v) via f16-ULP bitcast.
                m16 = small_pool.tile([128, KN], f16, tag="m16")
                nc.vector.tensor_scalar(out=m16[:], in0=v16[:], scalar1=0.0,
                                        scalar2=None,
                                        op0=mybir.AluOpType.is_gt)
                negv0 = small_pool.tile([128, 1], f32, tag="negv0")
                nc.vector.tensor_scalar(out=negv0[:], in0=v16[:, 0:1],
                                        scalar1=-1.0, scalar2=None,
                                        op0=mybir.AluOpType.mult)
                pfix = small_pool.tile([128, KN], f16, tag="pfix")
                nc.vector.tensor_scalar(out=pfix[:], in0=m16[:], scalar1=1.0,
                                        scalar2=negv0[:, 0:1],
                                        op0=mybir.AluOpType.subtract,
                                        op1=mybir.AluOpType.mult)
                vfix = small_pool.tile([128, KN], f16, tag="vfix")
                nc.vector.tensor_tensor(out=vfix[:], in0=v16[:], in1=m16[:],
                                        op=mybir.AluOpType.mult)
                nc.vector.tensor_tensor(out=vfix[:], in0=vfix[:], in1=pfix[:],
                                        op=mybir.AluOpType.add)
                bits = small_pool.tile([128, KN], f32, tag="bits")
                nc.vector.tensor_copy(bits[:], vfix[:].bitcast(u16))
                idxi = small_pool.tile([128, KN], i32, tag="idxi")
                nc.vector.tensor_scalar(out=idxi[:], in0=bits[:],
                                        scalar1=-1.0, scalar2=float(NJ_BASE),
                                        op0=mybir.AluOpType.mult,
                                        op1=mybir.AluOpType.add)

                own = small_pool.tile([128, CP], f16, tag="own")
                nc.sync.dma_start(own[:], own_d[blk * 128:(blk + 1) * 128, :])
                G = gat_pool.tile([128, KN, CP], f16, tag="G")
                if BATCH_GATHER:
                    nc.gpsimd.indirect_dma_start(
                        G[:, :, :], None, maskp_d[:],
                        bass.IndirectOffsetOnAxis(ap=idxi[:, 0:KN], axis=0))
                else:
                    for s in range(KN):
                        nc.gpsimd.indirect_dma_start(
                            G[:, s, :], None, maskp_d[:],
                            bass.IndirectOffsetOnAxis(ap=idxi[:, s:s + 1],
                                                      axis=0))

                flush_prev()
                prev = (G, own, blk)
            flush_prev()

            nc.sync.dma_start(partial_d[:], acc_all[:])

    nc.compile()
    return nc


def _get_program():
    global _PROGRAM
    if _PROGRAM is None:
        _PROGRAM = _build_program()
    return _PROGRAM


try:
    import ml_dtypes
    _BF = ml_dtypes.bfloat16
except ImportError:
    _BF = None


def _split3(v):
    """f32 -> (hi, mid, lo) bf16 triplet with hi+mid+lo ~ v to ~2^-25 rel."""
    v = np.asarray(v, np.float32)
    h = v.astype(_BF)
    r = v - h.astype(np.float32)
    m = r.astype(_BF)
    l = (r - m.astype(np.float32)).astype(_BF)
    return h, m, l


def _make_in_maps(pc: np.ndarray, mask: np.ndarray):
    pc = np.asarray(pc, np.float32)
    mask = np.asarray(mask, np.float32)
    nj = (NJ_BASE - np.arange(N)).astype(np.uint16).view(np.float16)
    in_maps = []
    for core in range(NCORES):
        b, h = divmod(core, 2)
        rows = slice(h * ROWS, (h + 1) * ROWS)
        pcb = pc[b]                       # (N, 3)
        sq = np.sum(pcb * pcb, axis=1)    # (N,)
        # 3-way bf16 split of 2*pc_n (rows) and pc_j (cols); P4 accumulates
        # the 6 dominant cross products + split -sq_j rows in f32 PSUM.
        xh, xm, xl = _split3(2.0 * pcb[rows])
        yh, ym, yl = _split3(pcb)
        sh, sm, sl = _split3(sq)
        ones = np.ones((ROWS,), _BF)
        lhsT = np.stack([r for a, _ in ((xh, yh), (xh, ym), (xm, yh),
                                        (xh, yl), (xl, yh), (xm, ym))
                         for r in (a[:, 0], a[:, 1], a[:, 2])]
                        + [ones, ones, ones], axis=0)
        rhs = np.stack([r for _, bb in ((xh, yh), (xh, ym), (xm, yh),
                                        (xh, yl), (xl, yh), (xm, ym))
                        for r in (bb[:, 0], bb[:, 1], bb[:, 2])]
                       + [-sh, -sm, -sl], axis=0)
        nthr = (R2 - sq[rows]).reshape(NBLK, 128).T.copy()
        maskp = np.zeros((N, CP), np.float16)
        maskp[:, :C] = mask[b].astype(np.float16)
        ownp = np.zeros((ROWS, CP), np.float16)
        ownp[:, :C] = mask[b][rows].astype(np.float16)
        in_maps.append({"lhsT": np.ascontiguousarray(lhsT),
                        "rhs": np.ascontiguousarray(rhs),
                        "nthr": np.ascontiguousarray(nthr),
                        "nj": nj,
                        "maskp": maskp,
                        "own": ownp})
    return in_maps


def _run(pc, mask, trace=False):
    nc = _get_program()
    in_maps = _make_in_maps(pc, mask)
    res = run_bass_kernel_spmd(nc, in_maps, list(range(NCORES)), trace=trace)
    total = sum(float(r["partial"].astype(np.float64).sum())
                for r in res.results)
    loss = np.float32(total / (B * N * KN))
    return np.asarray(loss, dtype=np.float32), res


def kernel(pc, mask):
    loss, _ = _run(pc, mask)
    return loss


# revision 7
# speedup vs baseline: 3.5185x; 2.9684x over previous
"""Trainium2 Bass kernel for nn_BallQLoss — V2: k-d-binned candidate pruning.

Host side: per core (= one (batch, half) shard), recursively median-split the
batch's 4096 points into 32 spatial leaves of 128 queries (the core takes 16
leaves = 2048 rows). For each 128-row leaf, the candidate set = all points
within the leaf bbox dilated by the ball radius (superset of every row's
in-ball set, so the device-side first-16-by-index selection stays EXACT).
Leaves are sorted by candidate count and assigned to program slots; slot
widths are the max across cores (envelope), so one SPMD program serves all
8 cores. Total scanned width drops ~5x vs the dense 4096.

Device per 128-row slot of width W:
  PE:   P4 = 2*dot(pc_n, pc_cand) - sq_cand     (K=21 bf16-split matmul)
  ACT:  S = sign(P4 + (r^2 - sq_n))             (one [128,W] PSUM read)
  DVE:  keyed = S * njb  (njb = f16 ULP-coded key of ORIGINAL point index)
        top-16 via max8 / (keyed < v8min)*keyed / max8
        pad fix: vfix = max(v16, (v16<=0)*v0); idx = NJ_BASE - bits(vfix)
  POOL: batched indirect gather mask rows -> G [128,16,32] f16
  DVE:  D = G - own; acc[:, slot] = sum |D|
Host: sum partial [128, NBLK] over cores, divide by B*N*K.
"""
import os
import sys

import numpy as np

try:
    import concourse.bass as bass
except ImportError:
    sys.path.insert(0, '/opt/trn_rl_repo')
    import concourse.bass as bass

import concourse.mybir as mybir
import concourse.tile as tile
from concourse import bacc, library_config
from concourse.bass_utils import run_bass_kernel_spmd

f32 = mybir.dt.float32
f16 = mybir.dt.float16
bf16 = mybir.dt.bfloat16
u16 = mybir.dt.uint16
i16 = mybir.dt.int16
i32 = mybir.dt.int32
KDIM = 21
NJ_BASE = 27648  # bits of f16(4096.0); key(j) = bitcast_f16(NJ_BASE - j)

B = 4
N = 4096
C = 30
CP = 32          # padded channels (64B gather rows)
KN = 16
RADIUS = np.float32(0.2)
R2 = RADIUS * RADIUS
NCORES = 8
ROWS = 2048
NBLK = ROWS // 128
WPAD = 256       # slot width granularity
MP = 128         # maskp row padding for 256B dma_gather descriptors
GATHER_STYLE = os.environ.get("GATHER_STYLE", "slot")  # "ant" | "slot"
WRAP_REPL = os.environ.get("WRAP_REPL", "1") == "1"
# wrap read AP: dst [16,128] i16; src elem offs = 16q + slot + 256g
WRAP_AP = [[16, 16], [1, 16], [256, 8]]

_PROGRAM = None  # (widths_key, nc)


def _build_program(widths):
    totw = int(sum(widths))
    wmax = int(max(widths))
    nc = bacc.Bacc("TRN2", target_bir_lowering=False, debug=False)

    lhsT_d = nc.dram_tensor("lhsT", [KDIM, ROWS], bf16, kind="ExternalInput")
    rhs_d = nc.dram_tensor("rhs", [KDIM, totw], bf16, kind="ExternalInput")
    nthr_d = nc.dram_tensor("nthr", [128, NBLK], f32, kind="ExternalInput")
    nj_d = nc.dram_tensor("nj", [totw], f16, kind="ExternalInput")
    maskp_d = nc.dram_tensor("maskp", [N, CP], f16, kind="ExternalInput")
    own_d = nc.dram_tensor("own", [ROWS, CP], f16, kind="ExternalInput")
    partial_d = nc.dram_tensor("partial", [128, NBLK], f32,
                               kind="ExternalOutput")

    with tile.TileContext(nc) as tc:
        with (
            tc.tile_pool(name="const", bufs=1) as const_pool,
            tc.tile_pool(name="psum", bufs=2, space="PSUM") as psum_pool,
            tc.tile_pool(name="sbS", bufs=2) as s_pool,
            tc.tile_pool(name="sbK", bufs=2) as k_pool,
            tc.tile_pool(name="sbK2", bufs=2) as k2_pool,
            tc.tile_pool(name="small", bufs=3) as small_pool,
            tc.tile_pool(name="gat", bufs=3) as gat_pool,
        ):
            lhsT = const_pool.tile([KDIM, ROWS], bf16)
            nc.sync.dma_start(lhsT[:], lhsT_d[:])
            rhs = const_pool.tile([KDIM, totw], bf16)
            nc.sync.dma_start(rhs[:], rhs_d[:])
            nthr = const_pool.tile([128, NBLK], f32)
            nc.sync.dma_start(nthr[:], nthr_d[:])
            nj = const_pool.tile([128, totw], f16)
            nc.sync.dma_start(nj[:], bass.AP(nj_d, 0, [[0, 128], [1, totw]]))
            acc_all = const_pool.tile([128, NBLK], f32)

            prev = None

            def flush_prev():
                nonlocal prev
                if prev is None:
                    return
                G, own, pblk = prev
                D = gat_pool.tile([128, KN, C], f16, tag="D")
                nc.vector.tensor_tensor(
                    out=D[:], in0=G[:, :, 0:C],
                    in1=own[:, 0:C].unsqueeze(1).broadcast_to((128, KN, C)),
                    op=mybir.AluOpType.subtract)
                nc.vector.reduce_sum(acc_all[:, pblk:pblk + 1], D[:],
                                     mybir.AxisListType.XY,
                                     apply_absolute_value=True)
                prev = None

            col = 0
            for blk in range(NBLK):
                W = int(widths[blk])
                S = s_pool.tile([128, wmax], f16, tag="S")
                keyed = k_pool.tile([128, wmax], f16, tag="K")
                p = psum_pool.tile([128, min(wmax, 2048)], f32, tag="p")
                nmm = (W + 511) // 512
                for f in range(nmm):
                    fs = slice(f * 512, min((f + 1) * 512, W))
                    nc.tensor.matmul(p[:, fs],
                                     lhsT[:, blk * 128:(blk + 1) * 128],
                                     rhs[:, col + f * 512:col + fs.stop])
                nc.scalar.activation(S[:, 0:W], p[:, 0:W],
                                     mybir.ActivationFunctionType.Sign,
                                     bias=nthr[:, blk:blk + 1], scale=1.0)
                nc.vector.tensor_tensor(out=keyed[:, 0:W], in0=S[:, 0:W],
                                        in1=nj[:, col:col + W],
                                        op=mybir.AluOpType.mult)

                v16 = small_pool.tile([128, KN], f16, tag="v16")
                nc.vector.max(v16[:, 0:8], keyed[:, 0:W])
                keyed2 = k2_pool.tile([128, wmax], f16, tag="K2")
                nc.vector.scalar_tensor_tensor(
                    out=keyed2[:, 0:W], in0=keyed[:, 0:W],
                    scalar=v16[:, 7:8], in1=keyed[:, 0:W],
                    op0=mybir.AluOpType.is_lt, op1=mybir.AluOpType.mult)
                nc.vector.max(v16[:, 8:16], keyed2[:, 0:W])

                # pad fix: vfix = max(v16, (v16<=0)*v0); all valid keys are in
                # (0, v0], invalid slots hold values <= 0.
                pfix = small_pool.tile([128, KN], f16, tag="pfix")
                nc.vector.scalar_tensor_tensor(
                    out=pfix[:], in0=v16[:], scalar=0.0,
                    in1=v16[:, 0:1].broadcast_to((128, KN)),
                    op0=mybir.AluOpType.is_le, op1=mybir.AluOpType.mult)
                vfix = small_pool.tile([128, KN], f16, tag="vfix")
                nc.vector.tensor_tensor(out=vfix[:], in0=v16[:], in1=pfix[:],
                                        op=mybir.AluOpType.max)
                bits = small_pool.tile([128, KN], f32, tag="bits")
                nc.vector.tensor_copy(bits[:], vfix[:].bitcast(u16))
                idxi = small_pool.tile([128, KN], i32, tag="idxi")
                nc.vector.tensor_scalar(out=idxi[:], in0=bits[:],
                                        scalar1=-1.0, scalar2=float(NJ_BASE),
                                        op0=mybir.AluOpType.mult,
                                        op1=mybir.AluOpType.add)

                own = small_pool.tile([128, CP], f16, tag="own")
                nc.sync.dma_start(own[:], own_d[blk * 128:(blk + 1) * 128, :])
                G = gat_pool.tile([128, KN, CP], f16, tag="G")
                for s in range(KN):
                    nc.gpsimd.indirect_dma_start(
                        G[:, s, :], None, maskp_d[:],
                        bass.IndirectOffsetOnAxis(ap=idxi[:, s:s + 1],
                                                  axis=0))

                flush_prev()
                prev = (G, own, blk)
                col += W
            flush_prev()

            nc.sync.dma_start(partial_d[:], acc_all[:])

    nc.compile()
    return nc


def _split3(v):
    import ml_dtypes
    BF = ml_dtypes.bfloat16
    v = np.asarray(v, np.float32)
    h = v.astype(BF)
    r = v - h.astype(np.float32)
    m = r.astype(BF)
    l = (r - m.astype(np.float32)).astype(BF)
    return h, m, l


def _kd_leaves(p, depth=5):
    """Recursive widest-axis median split; returns list of index arrays."""
    def rec(idx, d):
        if d == 0:
            return [idx]
        pts = p[idx]
        ax = int(np.argmax(pts.max(0) - pts.min(0)))
        o = np.argsort(pts[:, ax], kind='stable')
        half = len(idx) // 2
        return rec(idx[o[:half]], d - 1) + rec(idx[o[half:]], d - 1)
    return rec(np.arange(len(p), dtype=np.int64), depth)


def _plan(pc):
    """Per-core leaf order + candidate lists; returns (plans, widths).

    plans[core] = list of NBLK (rows_idx, cand_idx) in slot order.
    widths[slot] = envelope width (max candidate count over cores, padded).
    """
    pc = np.asarray(pc, np.float32)
    margin = np.float32(1e-5)
    plans = []
    for core in range(NCORES):
        b, h = divmod(core, 2)
        p = pc[b]
        leaves = _kd_leaves(p, 5)[h * NBLK:(h + 1) * NBLK]
        entries = []
        for rows_idx in leaves:
            q = p[rows_idx]
            lo = q.min(0) - RADIUS - margin
            hi = q.max(0) + RADIUS + margin
            cand = np.nonzero(np.all((p >= lo) & (p <= hi), axis=1))[0]
            entries.append((rows_idx, cand))
        entries.sort(key=lambda e: -len(e[1]))
        plans.append(entries)
    counts = np.array([[len(e[1]) for e in plan] for plan in plans])
    widths = ((counts.max(axis=0) + WPAD - 1) // WPAD) * WPAD
    return plans, widths


def _make_in_maps(pc, mask, plans, widths):
    pc = np.asarray(pc, np.float32)
    mask = np.asarray(mask, np.float32)
    totw = int(widths.sum())
    in_maps = []
    for core in range(NCORES):
        b, _ = divmod(core, 2)
        p = pc[b]
        sq = np.sum(p * p, axis=1)
        rows_perm = np.concatenate([e[0] for e in plans[core]])
        # candidate columns, padded with a far-away dummy point
        cand_cols = np.full((totw,), -1, np.int64)
        col = 0
        for slot, (rows_idx, cand) in enumerate(plans[core]):
            cand_cols[col:col + len(cand)] = cand
            col += int(widths[slot])
        valid = cand_cols >= 0
        pcc = np.where(valid[:, None], p[np.maximum(cand_cols, 0)], 1.0e3)
        sqc = (pcc * pcc).sum(1)

        xh, xm, xl = _split3(2.0 * p[rows_perm])
        yh, ym, yl = _split3(pcc)
        sh, sm, sl = _split3(sqc)
        import ml_dtypes
        ones = np.ones((ROWS,), ml_dtypes.bfloat16)
        lhsT = np.stack([r for a, _ in ((xh, yh), (xh, ym), (xm, yh),
                                        (xh, yl), (xl, yh), (xm, ym))
                         for r in (a[:, 0], a[:, 1], a[:, 2])]
                        + [ones, ones, ones], axis=0)
        rhs = np.stack([r for _, bb in ((xh, yh), (xh, ym), (xm, yh),
                                        (xh, yl), (xl, yh), (xm, ym))
                        for r in (bb[:, 0], bb[:, 1], bb[:, 2])]
                       + [-sh, -sm, -sl], axis=0)
        nthr = (R2 - sq[rows_perm]).reshape(NBLK, 128).T.copy()
        njc = (NJ_BASE - np.maximum(cand_cols, 0)).astype(np.uint16)
        nj = njc.view(np.float16).copy()
        maskp = np.zeros((N, CP), np.float16)
        maskp[:, :C] = mask[b].astype(np.float16)
        ownp = np.zeros((ROWS, CP), np.float16)
        ownp[:, :C] = mask[b][rows_perm].astype(np.float16)
        in_maps.append({"lhsT": np.ascontiguousarray(lhsT),
                        "rhs": np.ascontiguousarray(rhs),
                        "nthr": np.ascontiguousarray(nthr),
                        "nj": nj,
                        "maskp": maskp,
                        "own": ownp})
    return in_maps


def _get_program(widths):
    global _PROGRAM
    key = tuple(int(w) for w in widths)
    if _PROGRAM is None or _PROGRAM[0] != key:
        _PROGRAM = (key, _build_program(widths))
    return _PROGRAM[1]


def _run(pc, mask, trace=False):
    plans, widths = _plan(pc)
    nc = _get_program(widths)
    in_maps = _make_in_maps(pc, mask, plans, widths)
    res = run_bass_kernel_spmd(nc, in_maps, list(range(NCORES)), trace=trace)
    total = sum(float(r["partial"].astype(np.float64).sum())
                for r in res.results)
    loss = np.float32(total / (B * N * KN))
    return np.asarray(loss, dtype=np.float32), res


def kernel(pc, mask):
    loss, _ = _run(pc, mask)
    return loss
